# revision 5
# baseline (speedup 1.0000x reference)
"""DropDim kernel for Trainium2: out[b, t, d] = h[b, t, d] * mask[b, d].

h: (16, 4096, 1024) f32, mask: (16, 1024) bool. Pure data parallel over the
batch axis: each of the 8 NeuronCores handles 2 samples (32 MiB in / 32 MiB
out) — a memory-bound stream of load -> DVE multiply (in place) -> store.

Per core the 2-sample shard is viewed as [8192, 1024] row-major f32 and cut
into 8 tiles of [128 partitions, 8 rows, 1024] (4 MiB each, fully contiguous
per partition => near-peak DMA). The sample's mask row is DMA-broadcast once
to all 128 partitions and the multiply reads it with a stride-0 middle dim.
"""

import numpy as np

B, T, D = 16, 4096, 1024
N_CORES = 8
B_PER = B // N_CORES          # samples per core
ROWS_PER_CORE = B_PER * T     # 8192
R = int(__import__("os").environ.get("DROPDIM_R", "8"))  # rows per partition per tile
BUFS = int(__import__("os").environ.get("DROPDIM_BUFS", "5"))
MASK_MODE = __import__("os").environ.get("DROPDIM_MASK", "dma")  # dma | psum
TILE_ROWS = 128 * R           # rows per tile
N_TILES = ROWS_PER_CORE // TILE_ROWS
TILES_PER_SAMPLE = T // TILE_ROWS

_cache = {}


def _build_program():
    import concourse.tile as tile
    from concourse import bacc, mybir

    nc = bacc.Bacc(
        "TRN2",
        target_bir_lowering=False,
        debug=False,
        enable_asserts=False,
        num_devices=N_CORES,
    )
    h = nc.dram_tensor("h", [ROWS_PER_CORE, D], mybir.dt.float32, kind="ExternalInput")
    m = nc.dram_tensor("mask", [B_PER, D], mybir.dt.float32, kind="ExternalInput")
    o = nc.dram_tensor("out", [ROWS_PER_CORE, D], mybir.dt.float32, kind="ExternalOutput")

    with tile.TileContext(nc) as tc:
        with (
            tc.tile_pool(name="mask", bufs=1) as mpool,
            tc.tile_pool(name="data", bufs=BUFS) as dpool,
        ):
            # Mask broadcasts go on the scalar (ACT) HWDGE ring: the sync ring
            # carries the h loads and must not stall behind these at startup.
            mtiles = []
            if MASK_MODE == "psum":
                # Broadcast via the idle TensorEngine: ones[1,128].T @ mask[1,D]
                # lands the [128, D] mask in PSUM without touching the SBUF
                # fabric ports that the h stream saturates.
                with tc.tile_pool(name="mpsum", bufs=1, space="PSUM") as ppool:
                    ones = mpool.tile([1, 128], mybir.dt.float32, tag="ones")
                    nc.any.memset(ones[:], 1.0)
                    rows = mpool.tile([1, B_PER * D], mybir.dt.float32, tag="mrows")
                    nc.scalar.dma_start(out=rows[:], in_=m.ap().rearrange("s d -> (s d)").unsqueeze(0))
                    for s in range(B_PER):
                        mt = ppool.tile([128, D], mybir.dt.float32, tag=f"maskp{s}")
                        for j in range(0, D, 512):
                            nc.tensor.matmul(
                                mt[:, j : j + 512],
                                ones[:],
                                rows[:, s * D + j : s * D + j + 512],
                                start=True,
                                stop=True,
                            )
                        mtiles.append(mt)
                    run_body(nc, tc, dpool, mtiles, h, o)
            else:
                for s in range(B_PER):
                    mt = mpool.tile([128, D], mybir.dt.float32, tag=f"mask{s}")
                    nc.scalar.dma_start(out=mt[:], in_=m.ap()[s : s + 1, :].to_broadcast((128, D)))
                    mtiles.append(mt)
            for k in range(N_TILES):
                s = k // TILES_PER_SAMPLE
                t = dpool.tile([128, R, D], mybir.dt.float32)
                src = h.ap()[k * TILE_ROWS : (k + 1) * TILE_ROWS, :].rearrange(
                    "(p n) d -> p n d", p=128
                )
                nc.sync.dma_start(out=t[:], in_=src)
                mb = mtiles[s][:].unsqueeze(1).to_broadcast((128, R, D))
                nc.vector.tensor_mul(t[:], t[:], mb)
                dst = o.ap()[k * TILE_ROWS : (k + 1) * TILE_ROWS, :].rearrange(
                    "(p n) d -> p n d", p=128
                )
                nc.scalar.dma_start(out=dst, in_=t[:])
    nc.compile()
    return nc


def _get_program():
    if "nc" not in _cache:
        _cache["nc"] = _build_program()
    return _cache["nc"]


def make_in_maps(h, mask):
    h = np.ascontiguousarray(h, dtype=np.float32)
    mask_f = np.ascontiguousarray(mask).astype(np.float32)
    in_maps = []
    for c in range(N_CORES):
        in_maps.append(
            {
                "h": h[c * B_PER : (c + 1) * B_PER].reshape(ROWS_PER_CORE, D),
                "mask": mask_f[c * B_PER : (c + 1) * B_PER],
            }
        )
    return in_maps


def gather_out(results):
    out = np.empty((B, T, D), dtype=np.float32)
    for c in range(N_CORES):
        out[c * B_PER : (c + 1) * B_PER] = results[c]["out"].reshape(B_PER, T, D)
    return out


def kernel(h, mask):
    from concourse import bass_utils

    nc = _get_program()
    res = bass_utils.run_bass_kernel_spmd(nc, make_in_maps(h, mask), core_ids=list(range(N_CORES)))
    return gather_out(res.results)


# revision 8
# speedup vs baseline: 1.0833x; 1.0833x over previous
"""DropDim kernel for Trainium2: out[b, t, d] = h[b, t, d] * mask[b, d].

h: (16, 4096, 1024) f32, mask: (16, 1024) bool. Pure data parallel over the
batch axis: each of the 8 NeuronCores handles 2 samples (32 MiB in / 32 MiB
out) — a memory-bound stream of load -> DVE multiply (in place) -> store.

Per core the 2-sample shard is viewed as [8192, 1024] row-major f32 and cut
into 8 tiles of [128 partitions, 8 rows, 1024] (4 MiB each, fully contiguous
per partition => near-peak DMA). The sample's mask row is DMA-broadcast once
to all 128 partitions and the multiply reads it with a stride-0 middle dim.
"""

import numpy as np

B, T, D = 16, 4096, 1024
N_CORES = 8
B_PER = B // N_CORES          # samples per core
ROWS_PER_CORE = B_PER * T     # 8192
R = int(__import__("os").environ.get("DROPDIM_R", "8"))  # rows per partition per tile
BUFS = int(__import__("os").environ.get("DROPDIM_BUFS", "5"))
MASK_MODE = __import__("os").environ.get("DROPDIM_MASK", "dma")  # dma | psum
TILE_ROWS = 128 * R           # rows per tile
N_TILES = ROWS_PER_CORE // TILE_ROWS
TILES_PER_SAMPLE = T // TILE_ROWS

_cache = {}


def run_body(nc, tc, dpool, mtiles, h, o):
    from concourse import mybir

    for k in range(N_TILES):
        s = k // TILES_PER_SAMPLE
        t = dpool.tile([128, R, D], mybir.dt.float32)
        src = h.ap()[k * TILE_ROWS : (k + 1) * TILE_ROWS, :].rearrange(
            "(p n) d -> p n d", p=128
        )
        nc.sync.dma_start(out=t[:], in_=src)
        mb = mtiles[s][:].unsqueeze(1).to_broadcast((128, R, D))
        nc.vector.tensor_mul(t[:], t[:], mb)
        dst = o.ap()[k * TILE_ROWS : (k + 1) * TILE_ROWS, :].rearrange(
            "(p n) d -> p n d", p=128
        )
        nc.scalar.dma_start(out=dst, in_=t[:])


def _build_program():
    import concourse.tile as tile
    from concourse import bacc, mybir

    nc = bacc.Bacc(
        "TRN2",
        target_bir_lowering=False,
        debug=False,
        enable_asserts=False,
        num_devices=N_CORES,
    )
    h = nc.dram_tensor("h", [ROWS_PER_CORE, D], mybir.dt.float32, kind="ExternalInput")
    m = nc.dram_tensor("mask", [B_PER, D], mybir.dt.float32, kind="ExternalInput")
    o = nc.dram_tensor("out", [ROWS_PER_CORE, D], mybir.dt.float32, kind="ExternalOutput")

    with tile.TileContext(nc) as tc:
        with (
            tc.tile_pool(name="mask", bufs=1) as mpool,
            tc.tile_pool(name="data", bufs=BUFS) as dpool,
        ):
            # Mask broadcasts go on the scalar (ACT) HWDGE ring: the sync ring
            # carries the h loads and must not stall behind these at startup.
            mtiles = []
            if MASK_MODE == "psum":
                # Broadcast via the idle TensorEngine: ones[1,128].T @ mask[1,D]
                # lands the [128, D] mask in PSUM without touching the SBUF
                # fabric ports that the h stream saturates.
                with tc.tile_pool(name="mpsum", bufs=1, space="PSUM") as ppool:
                    ones = mpool.tile([1, 128], mybir.dt.float32, tag="ones")
                    nc.any.memset(ones[:], 1.0)
                    rows = mpool.tile([1, B_PER * D], mybir.dt.float32, tag="mrows")
                    nc.scalar.dma_start(out=rows[:], in_=m.ap().rearrange("s d -> (s d)").unsqueeze(0))
                    for s in range(B_PER):
                        mt = ppool.tile([128, D], mybir.dt.float32, tag=f"maskp{s}")
                        for j in range(0, D, 512):
                            nc.tensor.matmul(
                                mt[:, j : j + 512],
                                ones[:],
                                rows[:, s * D + j : s * D + j + 512],
                                start=True,
                                stop=True,
                            )
                        mtiles.append(mt)
                    run_body(nc, tc, dpool, mtiles, h, o)
            elif MASK_MODE == "gps":
                # 8 KiB DMA of both mask rows to partition 0, then GpSimd
                # partition_broadcast (POOL fabric, off the saturated DMA ports).
                rows = mpool.tile([1, B_PER * D], mybir.dt.float32, tag="mrows")
                nc.scalar.dma_start(out=rows[:], in_=m.ap().rearrange("s d -> (s d)").unsqueeze(0))
                for s in range(B_PER):
                    mt = mpool.tile([128, D], mybir.dt.float32, tag=f"mask{s}")
                    nc.gpsimd.partition_broadcast(mt[:], rows[0:1, s * D : (s + 1) * D])
                    mtiles.append(mt)
                run_body(nc, tc, dpool, mtiles, h, o)
            else:
                for s in range(B_PER):
                    mt = mpool.tile([128, D], mybir.dt.float32, tag=f"mask{s}")
                    nc.scalar.dma_start(out=mt[:], in_=m.ap()[s : s + 1, :].to_broadcast((128, D)))
                    mtiles.append(mt)
                run_body(nc, tc, dpool, mtiles, h, o)
    nc.compile()
    return nc


def _get_program():
    if "nc" not in _cache:
        _cache["nc"] = _build_program()
    return _cache["nc"]


def make_in_maps(h, mask):
    h = np.ascontiguousarray(h, dtype=np.float32)
    mask_f = np.ascontiguousarray(mask).astype(np.float32)
    in_maps = []
    for c in range(N_CORES):
        in_maps.append(
            {
                "h": h[c * B_PER : (c + 1) * B_PER].reshape(ROWS_PER_CORE, D),
                "mask": mask_f[c * B_PER : (c + 1) * B_PER],
            }
        )
    return in_maps


def gather_out(results):
    out = np.empty((B, T, D), dtype=np.float32)
    for c in range(N_CORES):
        out[c * B_PER : (c + 1) * B_PER] = results[c]["out"].reshape(B_PER, T, D)
    return out


def kernel(h, mask):
    from concourse import bass_utils

    nc = _get_program()
    res = bass_utils.run_bass_kernel_spmd(nc, make_in_maps(h, mask), core_ids=list(range(N_CORES)))
    return gather_out(res.results)


# revision 9
# speedup vs baseline: 1.1887x; 1.0972x over previous
"""DropDim kernel for Trainium2: out[b, t, d] = h[b, t, d] * mask[b, d].

h: (16, 4096, 1024) f32, mask: (16, 1024) bool. Pure data parallel over the
batch axis: each of the 8 NeuronCores handles 2 samples (32 MiB in / 32 MiB
out) — a memory-bound stream of load -> DVE multiply (in place) -> store.

Per core the 2-sample shard is viewed as [8192, 1024] row-major f32 and cut
into 8 tiles of [128 partitions, 8 rows, 1024] (4 MiB each, fully contiguous
per partition => near-peak DMA). The sample's mask row is DMA-broadcast once
to all 128 partitions and the multiply reads it with a stride-0 middle dim.
Loads ride the sync HWDGE ring, stores + mask broadcasts the scalar ring;
5 tile buffers keep the DMA stream packed (measured ~99.9% DMA-active,
~413 GB/s of the 435 GB/s SBUF-fabric ceiling).
"""

import numpy as np

B, T, D = 16, 4096, 1024
N_CORES = 8
B_PER = B // N_CORES          # samples per core
ROWS_PER_CORE = B_PER * T     # 8192
R = 8                         # rows per partition per tile
BUFS = 5
TILE_ROWS = 128 * R           # 1024 rows per tile
N_TILES = ROWS_PER_CORE // TILE_ROWS      # 8
TILES_PER_SAMPLE = T // TILE_ROWS         # 4

_cache = {}


def _build_program():
    import concourse.tile as tile
    from concourse import bacc, mybir

    nc = bacc.Bacc(
        "TRN2",
        target_bir_lowering=False,
        debug=False,
        enable_asserts=False,
        num_devices=N_CORES,
    )
    h = nc.dram_tensor("h", [ROWS_PER_CORE, D], mybir.dt.float32, kind="ExternalInput")
    m = nc.dram_tensor("mask", [B_PER, D], mybir.dt.float32, kind="ExternalInput")
    o = nc.dram_tensor("out", [ROWS_PER_CORE, D], mybir.dt.float32, kind="ExternalOutput")

    with tile.TileContext(nc) as tc:
        with (
            tc.tile_pool(name="mask", bufs=1) as mpool,
            tc.tile_pool(name="data", bufs=BUFS) as dpool,
        ):
            # Mask broadcasts go on the scalar (ACT) HWDGE ring: the sync ring
            # carries the h loads and must not stall behind these at startup.
            mtiles = []
            for s in range(B_PER):
                mt = mpool.tile([128, D], mybir.dt.float32, tag=f"mask{s}")
                nc.scalar.dma_start(out=mt[:], in_=m.ap()[s : s + 1, :].to_broadcast((128, D)))
                mtiles.append(mt)
            for k in range(N_TILES):
                s = k // TILES_PER_SAMPLE
                t = dpool.tile([128, R, D], mybir.dt.float32)
                src = h.ap()[k * TILE_ROWS : (k + 1) * TILE_ROWS, :].rearrange(
                    "(p n) d -> p n d", p=128
                )
                nc.sync.dma_start(out=t[:], in_=src)
                mb = mtiles[s][:].unsqueeze(1).to_broadcast((128, R, D))
                nc.vector.tensor_mul(t[:], t[:], mb)
                dst = o.ap()[k * TILE_ROWS : (k + 1) * TILE_ROWS, :].rearrange(
                    "(p n) d -> p n d", p=128
                )
                nc.scalar.dma_start(out=dst, in_=t[:])
    nc.compile()
    return nc


def _get_program():
    if "nc" not in _cache:
        _cache["nc"] = _build_program()
    return _cache["nc"]


def make_in_maps(h, mask):
    h = np.ascontiguousarray(h, dtype=np.float32)
    mask_f = np.ascontiguousarray(mask).astype(np.float32)
    in_maps = []
    for c in range(N_CORES):
        in_maps.append(
            {
                "h": h[c * B_PER : (c + 1) * B_PER].reshape(ROWS_PER_CORE, D),
                "mask": mask_f[c * B_PER : (c + 1) * B_PER],
            }
        )
    return in_maps


def gather_out(results):
    out = np.empty((B, T, D), dtype=np.float32)
    for c in range(N_CORES):
        out[c * B_PER : (c + 1) * B_PER] = results[c]["out"].reshape(B_PER, T, D)
    return out


def kernel(h, mask):
    from concourse import bass_utils

    nc = _get_program()
    res = bass_utils.run_bass_kernel_spmd(nc, make_in_maps(h, mask), core_ids=list(range(N_CORES)))
    return gather_out(res.results)


# revision 10
# speedup vs baseline: 1.1889x; 1.0002x over previous
"""DropDim kernel for Trainium2: out[b, t, d] = h[b, t, d] * mask[b, d].

h: (16, 4096, 1024) f32, mask: (16, 1024) bool. Pure data parallel over the
batch axis: each of the 8 NeuronCores handles 2 samples (32 MiB in / 32 MiB
out) — a memory-bound stream of load -> DVE multiply (in place) -> store.

Per core the 2-sample shard is viewed as [8192, 1024] row-major f32 and cut
into 8 tiles of [128 partitions, 8 rows, 1024] (4 MiB each, fully contiguous
per partition => near-peak DMA). The sample's mask row is DMA-broadcast once
to all 128 partitions and the multiply reads it with a stride-0 middle dim.
Loads ride the sync HWDGE ring, stores + mask broadcasts the scalar ring;
5 tile buffers keep the DMA stream packed (measured ~99.9% DMA-active,
~413 GB/s of the 435 GB/s SBUF-fabric ceiling).
"""

import numpy as np

B, T, D = 16, 4096, 1024
N_CORES = 8
B_PER = B // N_CORES          # samples per core
ROWS_PER_CORE = B_PER * T     # 8192
R = 8                         # rows per partition per tile
BUFS = 5
TILE_ROWS = 128 * R           # 1024 rows per tile
N_TILES = ROWS_PER_CORE // TILE_ROWS      # 8
TILES_PER_SAMPLE = T // TILE_ROWS         # 4

_cache = {}


def _build_program():
    import concourse.tile as tile
    from concourse import bacc, mybir

    nc = bacc.Bacc(
        "TRN2",
        target_bir_lowering=False,
        debug=False,
        enable_asserts=False,
        num_devices=N_CORES,
    )
    h = nc.dram_tensor("h", [ROWS_PER_CORE, D], mybir.dt.float32, kind="ExternalInput")
    m = nc.dram_tensor("mask", [B_PER, D], mybir.dt.float32, kind="ExternalInput")
    o = nc.dram_tensor("out", [ROWS_PER_CORE, D], mybir.dt.float32, kind="ExternalOutput")

    with tile.TileContext(nc) as tc:
        with (
            tc.tile_pool(name="mask", bufs=1) as mpool,
            tc.tile_pool(name="data", bufs=BUFS) as dpool,
        ):
            # Mask rows arrive as one 8 KiB DMA on the scalar (ACT) ring, then
            # GpSimd partition_broadcast fans them to all 128 partitions over
            # the POOL fabric — zero extra load on the saturated HBM/SDMA path.
            mtiles = []
            rows = mpool.tile([1, B_PER * D], mybir.dt.float32, tag="mrows")
            nc.scalar.dma_start(out=rows[:], in_=m.ap().rearrange("s d -> (s d)").unsqueeze(0))
            for s in range(B_PER):
                mt = mpool.tile([128, D], mybir.dt.float32, tag=f"mask{s}")
                nc.gpsimd.partition_broadcast(mt[:], rows[0:1, s * D : (s + 1) * D])
                mtiles.append(mt)
            for k in range(N_TILES):
                s = k // TILES_PER_SAMPLE
                t = dpool.tile([128, R, D], mybir.dt.float32)
                src = h.ap()[k * TILE_ROWS : (k + 1) * TILE_ROWS, :].rearrange(
                    "(p n) d -> p n d", p=128
                )
                nc.sync.dma_start(out=t[:], in_=src)
                mb = mtiles[s][:].unsqueeze(1).to_broadcast((128, R, D))
                nc.vector.tensor_mul(t[:], t[:], mb)
                dst = o.ap()[k * TILE_ROWS : (k + 1) * TILE_ROWS, :].rearrange(
                    "(p n) d -> p n d", p=128
                )
                nc.scalar.dma_start(out=dst, in_=t[:])
    nc.compile()
    return nc


def _get_program():
    if "nc" not in _cache:
        _cache["nc"] = _build_program()
    return _cache["nc"]


def make_in_maps(h, mask):
    h = np.ascontiguousarray(h, dtype=np.float32)
    mask_f = np.ascontiguousarray(mask).astype(np.float32)
    in_maps = []
    for c in range(N_CORES):
        in_maps.append(
            {
                "h": h[c * B_PER : (c + 1) * B_PER].reshape(ROWS_PER_CORE, D),
                "mask": mask_f[c * B_PER : (c + 1) * B_PER],
            }
        )
    return in_maps


def gather_out(results):
    out = np.empty((B, T, D), dtype=np.float32)
    for c in range(N_CORES):
        out[c * B_PER : (c + 1) * B_PER] = results[c]["out"].reshape(B_PER, T, D)
    return out


def kernel(h, mask):
    from concourse import bass_utils

    nc = _get_program()
    res = bass_utils.run_bass_kernel_spmd(nc, make_in_maps(h, mask), core_ids=list(range(N_CORES)))
    return gather_out(res.results)


# revision 12
# speedup vs baseline: 1.1931x; 1.0036x over previous
"""DropDim kernel for Trainium2: out[b, t, d] = h[b, t, d] * mask[b, d].

h: (16, 4096, 1024) f32, mask: (16, 1024) bool. Pure data parallel over the
batch axis: each of the 8 NeuronCores handles 2 samples (32 MiB in / 32 MiB
out) — a memory-bound stream of load -> DVE multiply (in place) -> store.

Per core the 2-sample shard is viewed as [8192, 1024] row-major f32 and cut
into 8 tiles of [128 partitions, 8 rows, 1024] (4 MiB each, fully contiguous
per partition => near-peak DMA). The sample's mask row is DMA-broadcast once
to all 128 partitions and the multiply reads it with a stride-0 middle dim.
Loads ride the sync HWDGE ring, stores + mask broadcasts the scalar ring;
5 tile buffers keep the DMA stream packed (measured ~99.9% DMA-active,
~413 GB/s of the 435 GB/s SBUF-fabric ceiling).
"""

import numpy as np

B, T, D = 16, 4096, 1024
N_CORES = 8
B_PER = B // N_CORES          # samples per core
ROWS_PER_CORE = B_PER * T     # 8192
R = 8                         # rows per partition per tile
BUFS = 5
TILE_ROWS = 128 * R           # 1024 rows per tile
N_TILES = ROWS_PER_CORE // TILE_ROWS      # 8
TILES_PER_SAMPLE = T // TILE_ROWS         # 4

_cache = {}


def _build_program():
    import concourse.tile as tile
    from concourse import bacc, mybir

    nc = bacc.Bacc(
        "TRN2",
        target_bir_lowering=False,
        debug=False,
        enable_asserts=False,
        num_devices=N_CORES,
    )
    h = nc.dram_tensor("h", [ROWS_PER_CORE, D], mybir.dt.float32, kind="ExternalInput")
    m = nc.dram_tensor("mask", [B_PER, D], mybir.dt.float32, kind="ExternalInput")
    o = nc.dram_tensor("out", [ROWS_PER_CORE, D], mybir.dt.float32, kind="ExternalOutput")

    with tile.TileContext(nc) as tc:
        with (
            tc.tile_pool(name="mask", bufs=1) as mpool,
            tc.tile_pool(name="data", bufs=BUFS) as dpool,
        ):
            # Mask rows arrive as one 8 KiB DMA on the scalar (ACT) ring, then
            # GpSimd partition_broadcast fans them to all 128 partitions over
            # the POOL fabric — zero extra load on the saturated HBM/SDMA path.
            mtiles = []
            rows = mpool.tile([1, B_PER * D], mybir.dt.float32, tag="mrows")
            nc.scalar.dma_start(out=rows[:], in_=m.ap().rearrange("s d -> (s d)").unsqueeze(0))
            for s in range(B_PER):
                mt = mpool.tile([128, D], mybir.dt.float32, tag=f"mask{s}")
                nc.gpsimd.partition_broadcast(mt[:], rows[0:1, s * D : (s + 1) * D])
                mtiles.append(mt)
            for k in range(N_TILES):
                s = k // TILES_PER_SAMPLE
                t = dpool.tile([128, R, D], mybir.dt.float32)
                src = h.ap()[k * TILE_ROWS : (k + 1) * TILE_ROWS, :].rearrange(
                    "(p n) d -> p n d", p=128
                )
                nc.sync.dma_start(out=t[:], in_=src)
                mb = mtiles[s][:].unsqueeze(1).to_broadcast((128, R, D))
                nc.vector.tensor_mul(t[:], t[:], mb)
                dst = o.ap()[k * TILE_ROWS : (k + 1) * TILE_ROWS, :].rearrange(
                    "(p n) d -> p n d", p=128
                )
                nc.scalar.dma_start(out=dst, in_=t[:])
    nc.compile()
    return nc


def _get_program():
    if "nc" not in _cache:
        _cache["nc"] = _build_program()
    return _cache["nc"]


def make_in_maps(h, mask):
    h = np.ascontiguousarray(h, dtype=np.float32)
    mask_f = np.ascontiguousarray(mask).astype(np.float32)
    in_maps = []
    for c in range(N_CORES):
        in_maps.append(
            {
                "h": h[c * B_PER : (c + 1) * B_PER].reshape(ROWS_PER_CORE, D),
                "mask": mask_f[c * B_PER : (c + 1) * B_PER],
            }
        )
    return in_maps


def gather_out(results):
    out = np.empty((B, T, D), dtype=np.float32)
    for c in range(N_CORES):
        out[c * B_PER : (c + 1) * B_PER] = results[c]["out"].reshape(B_PER, T, D)
    return out


def kernel(h, mask):
    from concourse import bass_utils

    nc = _get_program()
    res = bass_utils.run_bass_kernel_spmd(nc, make_in_maps(h, mask), core_ids=list(range(N_CORES)))
    return gather_out(res.results)
